# revision 23
# baseline (speedup 1.0000x reference)
"""Cross-attention kernel for one TRN2 chip (8 NeuronCores).

Sharding: core = (batch b in {0,1}) x (head-group of 4 heads).  Each core
computes attention for its 4 heads of its batch element and a partial output
projection [N, 1024]; the host sums the 4 partials per batch and adds bias.

Key structure (all matmuls bf16, fp32 PSUM):
  - x/ctx pre-transposed on host, cast-loaded bf16; DMA ordered so the
    first K projection can start after ~1.5MB has landed.
  - QK per m-tile: two concurrent row-tiled matmuls into one [128,1024]
    PSUM tile from a 3-deep pool so the PE runs 3 tiles ahead of exp and
    never drops out of the 2.4GHz p-state.
  - exp is split across THREE engines per 16-tile group: ScalarE native
    exp (9), DVE int16-Schraudolph->bf16 in one op (5), Pool(GpSimd)
    int16-Schraudolph (2).  AV consumes pT at lag 4 so even the slow
    Pool exp is ready in time.
  - AV is v-stationary: lhsT = v[m, d|ones], moving = pT[m,512]; output
    accumulates in PSUM in d-major [d, n] layout (no output transposes),
    with the ones column producing the softmax denominator in row 64.
  - Drain: two DVE copies PSUM->SBUF free the oT banks within ~1.5us;
    the normalization (reshape-DMA -> [128,8] reciprocal -> DRAM-bounce
    partition-broadcast -> two DVE mults -> head1 partition-shift DMA)
    is fully deferred off the critical path.
  - Output projection y = oTn.T @ wo per n-tile, streamed out per-tile.
"""

import dataclasses

import numpy as np

import concourse.bass as bass
import concourse.mybir as mybir
import concourse.tile as tile
from concourse import bacc
from concourse.bass import ts
from concourse.bass_utils import run_bass_kernel_spmd

B, N, M, C = 2, 2048, 2048, 1024
HEADS, DH = 16, 64
H_PER = 4                # heads per core
DHC = H_PER * DH         # 256: per-core slice of INNER
SCALE = DH ** -0.5
P = 128
NT = N // P              # 16 n-tiles
MT = M // P              # 16 m-tiles
CCH = C // P             # 8 contraction chunks
FD = 512                 # attention n-chunk (PSUM bank)
NJ = N // FD             # 4 n-chunks
N_CORES = 8
LAG = 5                  # av trails qk/exp by this many m-tiles

F32 = mybir.dt.float32
BF16 = mybir.dt.bfloat16
I16 = mybir.dt.int16
EXP = mybir.ActivationFunctionType.Exp
# int16 Schraudolph fast-exp: exp(x*SCALE) ~= bitcast_bf16(i16(x*KS + BS))
KS16 = SCALE * (1 << 7) / float(np.log(2.0))
BS16 = float(127 * (1 << 7)) - 366392.0 / 65536.0

_CACHE = {}


def _build():
    nc = bacc.Bacc("TRN2", target_bir_lowering=False, debug=False,
                   num_devices=N_CORES, num_swdge_queues=4)

    xT_d = nc.dram_tensor("xT", (C, N), BF16, kind="ExternalInput").ap()
    cT_d = nc.dram_tensor("cT", (C, M), BF16, kind="ExternalInput").ap()
    msk_d = nc.dram_tensor("msk", (M, 1), F32, kind="ExternalInput").ap()
    wq_d = nc.dram_tensor("wq", (C, DHC), BF16, kind="ExternalInput").ap()
    wk_d = nc.dram_tensor("wk", (C, DHC), BF16, kind="ExternalInput").ap()
    wv_d = nc.dram_tensor("wv", (C, DHC), BF16, kind="ExternalInput").ap()
    wo_d = nc.dram_tensor("wo", (DHC, C), BF16, kind="ExternalInput").ap()
    y_d = nc.dram_tensor("y", (N, C), BF16, kind="ExternalOutput").ap()

    with tile.TileContext(nc) as tc:
        with (
            tc.tile_pool(name="const", bufs=1) as const,
            tc.tile_pool(name="pTp", bufs=8) as pTp,
            tc.tile_pool(name="pTp2", bufs=8) as pTp2,
            tc.tile_pool(name="orp", bufs=3) as orp,
            tc.tile_pool(name="rbp", bufs=2) as rbp,
            tc.tile_pool(name="yp", bufs=3) as yp,
            tc.tile_pool(name="dramp", bufs=8, space="DRAM") as dramp,
        ):
            # ---- persistent SBUF tensors ----
            xT = const.tile([P, CCH, N], BF16, name="xT")
            cT = const.tile([P, CCH, M], BF16, name="cT")
            qT2 = const.tile([P, 2, N], BF16, name="qT2")
            kT2 = const.tile([P, 2, M], BF16, name="kT2")
            # v: [m-partition, m-tile, head, d(64)+ones(1)]
            v_sb = const.tile([P, MT, H_PER, DH + 1], BF16, name="v")
            wq_sb = const.tile([P, CCH, DHC], BF16, name="wq")
            wk_sb = const.tile([P, CCH, DHC], BF16, name="wk")
            wv_sb = const.tile([P, CCH, DHC], BF16, name="wv")
            wo_sb = const.tile([P, 2, C], BF16, name="wo")
            msk_sb = const.tile([P, MT, 1], F32, name="msk")
            oTn = const.tile([P, 2, N], BF16, name="oTn")

            # ---- input DMA, ordered by first use ----
            nc.sync.dma_start(
                out=wk_sb, in_=wk_d.rearrange("(cc p) d -> p cc d", p=P))
            nc.sync.dma_start(
                out=msk_sb, in_=msk_d.rearrange("(t p) o -> p t o", p=P))
            cTv = cT_d.rearrange("(cc p) n -> p cc n", p=P)
            xTv = xT_d.rearrange("(cc p) n -> p cc n", p=P)
            nc.sync.dma_start(
                out=cT[:, :, ts(0, FD)], in_=cTv[:, :, ts(0, FD)])
            nc.sync.dma_start(
                out=wv_sb, in_=wv_d.rearrange("(cc p) d -> p cc d", p=P))
            for g in range(1, 4):
                nc.sync.dma_start(
                    out=cT[:, :, ts(g, FD)], in_=cTv[:, :, ts(g, FD)])
            nc.sync.dma_start(
                out=wq_sb, in_=wq_d.rearrange("(cc p) d -> p cc d", p=P))
            for g in range(4):
                nc.sync.dma_start(
                    out=xT[:, :, ts(g, FD)], in_=xTv[:, :, ts(g, FD)])
            nc.sync.dma_start(
                out=wo_sb, in_=wo_d.rearrange("(dc p) e -> p dc e", p=P))

            nc.vector.memset(v_sb, 1.0)

            # shared PSUM pool: projections (phase A) + scores (phase B)
            ps_cm = tc.tile_pool(name="ps", bufs=3, space="PSUM")
            ps = ps_cm.__enter__()           # 3 x 2 banks

            def proj_T(w_sb, srcT, dstT2, dc, j, alt):
                pt = ps.tile([P, 2, FD], F32, name="ps")
                for cc in range(CCH):
                    nc.tensor.matmul(
                        pt[:, 0, :], lhsT=w_sb[:, cc, ts(dc, P)],
                        rhs=srcT[:, cc, ts(j, FD)],
                        start=(cc == 0), stop=(cc == CCH - 1))
                dst = dstT2[:, dc, ts(j, FD)]
                if alt:
                    nc.vector.tensor_copy(dst, pt[:, 0, :])
                else:
                    nc.scalar.copy(dst, pt[:, 0, :])

            def proj_V(m0):
                vp = ps.tile([P, 2, FD], F32, name="ps")
                vv = vp.rearrange("p mi (h d) -> p mi h d", h=H_PER * 2)
                for mi in range(2):
                    for cc in range(CCH):
                        nc.tensor.matmul(
                            vp[:, mi, 0:DHC],
                            lhsT=cT[:, cc, ts(m0 + mi, P)],
                            rhs=wv_sb[:, cc, :],
                            start=(cc == 0), stop=(cc == CCH - 1))
                nc.vector.tensor_copy(
                    v_sb[:, m0:m0 + 2, :, 0:DH],
                    vp[:, :, 0:DHC].rearrange("p mi (h d) -> p mi h d",
                                              h=H_PER))
                del vv
                for mi in range(2):
                    nc.vector.tensor_scalar_mul(
                        v_sb[:, m0 + mi, :, :], v_sb[:, m0 + mi, :, :],
                        msk_sb[:, m0 + mi, :])

            # ---- phase A: all K, all V, Q(dc0,0) ----
            alt = 0
            for g in range(4):
                proj_T(wk_sb, cT, kT2, 0, g, alt % 2)
                alt += 1
                proj_V(4 * g)
                proj_V(4 * g + 2)
            for g in range(4):
                proj_T(wk_sb, cT, kT2, 1, g, alt % 2)
                alt += 1
            proj_T(wq_sb, xT, qT2, 0, 0, alt % 2)
            alt += 1

            # ---- phase B: attention, with exp-free PE work (projections
            # and y-tiles) interleaved between groups so ScalarE/DVE get
            # catch-up windows and the PE never stalls ----
            ps_o_cm = tc.tile_pool(name="ps_o", bufs=1, space="PSUM")
            ps_o = ps_o_cm.__enter__()       # 2 x 1 bank oT accumulators

            def qk(sT, dc, j, m):
                for s in range(2):
                    nc.tensor.matmul(
                        sT[:, s, :],
                        lhsT=kT2[s * DH:(s + 1) * DH, dc, ts(m, P)],
                        rhs=qT2[s * DH:(s + 1) * DH, dc, ts(j, FD)],
                        start=True, stop=True)

            def av(oPs, pT, dc, m):
                # oT[d, n] += v[m, d|1].T @ pT[m, n]; stationary = v (65 col)
                for s in range(2):
                    nc.tensor.matmul(
                        oPs[s][0:DH + 1, :],
                        lhsT=v_sb[:, m, 2 * dc + s, :],
                        rhs=pT[:, s, :],
                        start=(m == 0), stop=(m == MT - 1),
                        skip_group_check=True)

            def normalize(o0, o1, dc, j):
                # denominators live in row 64 of o0/o1 as [1, 512] each:
                # reshape-DMA both to [128, 4]+[128, 4], fast reciprocal,
                # bounce through DRAM, broadcast-read, multiply (Pool).
                rden = rbp.tile([P, 8], F32, name="rden")
                for s, o in ((0, o0), (1, o1)):
                    sp = o[DH:DH + 1, :]
                    sp_r = dataclasses.replace(
                        sp, ap=[sp.ap[0], (4, P), (1, 4)])
                    nc.gpsimd.dma_start(out=rden[:, 4 * s:4 * s + 4],
                                        in_=sp_r)
                rrec = rbp.tile([P, 8], F32, name="rrec")
                nc.vector.reciprocal(rrec, rden)
                # store recips to DRAM s-major: scr[s*512 + p*4 + c]
                scr = dramp.tile([2, FD], F32, name="scr")
                sap = scr[:, :]
                s_out = dataclasses.replace(
                    sap, ap=[(4, P), (FD, 2), (1, 4)])
                nc.gpsimd.dma_start(
                    out=s_out,
                    in_=rrec[:, :].rearrange("p (s c) -> p s c", s=2))
                rcb = rbp.tile([P, 2, FD], F32, name="rcb")
                # broadcast-read: partitions x (s,n-hi fused) x n-lo
                bap = dataclasses.replace(
                    sap, ap=[(0, DH), (4, 2 * P), (1, 4)])
                oap = rcb[0:DH, :, :]
                oap = dataclasses.replace(
                    oap, ap=[oap.ap[0], (4, 2 * P), (1, 4)])
                nc.gpsimd.dma_start(out=oap, in_=bap)
                nc.gpsimd.tensor_mul(
                    oTn[0:DH, dc, ts(j, FD)], o0[0:DH, :],
                    rcb[0:DH, 0, :])
                o1b = rbp.tile([P, FD], BF16, name="o1b")
                nc.gpsimd.tensor_mul(
                    o1b[0:DH, :], o1[0:DH, :], rcb[0:DH, 1, :])
                nc.gpsimd.dma_start(
                    out=oTn[DH:P, dc, ts(j, FD)], in_=o1b[0:DH, :])

            def drain_copies(oPs, dc, j):
                o0 = orp.tile([P, FD], F32, name="o_raw0")
                o1 = orp.tile([P, FD], F32, name="o_raw1")
                nc.scalar.copy(o0[0:DH + 1, :], oPs[0][0:DH + 1, :])
                nc.vector.tensor_copy(o1[0:DH + 1, :], oPs[1][0:DH + 1, :])
                return (o0, o1, dc, j)

            def stage_drain(oPs, dc, j):
                pend["o0"] = orp.tile([P, FD], F32, name="o_raw0")
                pend["o1"] = orp.tile([P, FD], F32, name="o_raw1")
                pend["copies"] = (oPs, dc, j)

            def y_tile(i):
                y_ps = ps.tile([P, 2, FD], F32, name="ps")
                for col in range(2):
                    for dc in range(2):
                        nc.tensor.matmul(
                            y_ps[:, col, :],
                            lhsT=oTn[:, dc, ts(i, P)],
                            rhs=wo_sb[:, dc, ts(col, FD)],
                            start=(dc == 0), stop=(dc == 1))
                y_sb = yp.tile([P, C], BF16, name="ysb")
                if i % 2:
                    nc.vector.tensor_copy(
                        y_sb.rearrange("p (a b) -> p a b", a=2), y_ps)
                else:
                    nc.scalar.copy(
                        y_sb.rearrange("p (a b) -> p a b", a=2), y_ps)
                nc.sync.dma_start(out=y_d[ts(i, P), :], in_=y_sb)

            pend = {}
            alt2 = [0]

            def b_group(dc, j):
                oPs = [ps_o.tile([P, FD], F32, name=f"o{s}")
                       for s in range(2)]
                pTs = []
                for m in range(MT):
                    sT = ps.tile([P, 2, FD], F32, name="ps")
                    qk(sT, dc, j, m)
                    if m == 2 and "copies" in pend:
                        pend["copies2"] = pend.pop("copies")
                        nc.scalar.copy(pend["o0"][0:DH + 1, :],
                                       pend["copies2"][0][0][0:DH + 1, :])
                    if m == 3 and "copies2" in pend:
                        oPs2, dc2, j2 = pend.pop("copies2")
                        nc.vector.tensor_copy(pend["o1"][0:DH + 1, :],
                                              oPs2[1][0:DH + 1, :])
                        pend["norm"] = (pend.pop("o0"), pend.pop("o1"),
                                        dc2, j2)
                    if m == 8 and "norm" in pend:
                        normalize(*pend.pop("norm"))
                    if m >= LAG:
                        av(oPs, pTs[m - LAG], dc, m - LAG)
                    # whole-tile exp alternating engines; score slot frees
                    # ~1.3us after qk (ring deadline 1.9us at full clock)
                    pT = pTp.tile([P, 2, FD], BF16, name="pT")
                    if m % 2:
                        nc.vector.tensor_scalar(
                            pT.bitcast(I16)[:, :, :], sT, KS16, BS16,
                            op0=mybir.AluOpType.mult,
                            op1=mybir.AluOpType.add)
                    else:
                        nc.scalar.activation(pT, sT, EXP, scale=SCALE)
                    pTs.append(pT)
                for t in range(LAG):
                    av(oPs, pTs[MT - LAG + t], dc, MT - LAG + t)
                stage_drain(oPs, dc, j)

            # boundary packs: exactly 3 ps-ring allocations per pack so
            # the score ring phase is preserved; emitted after a group's
            # trailing avs (exp(m15) is done by then -> no slot stall)
            def pack(projs):
                for t in range(3):
                    if t < len(projs):
                        dcq, jq = projs[t]
                        proj_T(wq_sb, xT, qT2, dcq, jq, alt2[0] % 2)
                        alt2[0] += 1
                    else:
                        ps.tile([P, 2, FD], F32, name="ps")  # ring dummy

            packs = {0: [(1, 0), (0, 1), (1, 1)],
                     2: [(0, 2), (1, 2)],
                     4: [(0, 3), (1, 3)]}
            k = 0
            for j in range(NJ):
                for dc in range(2):
                    b_group(dc, j)
                    if k in packs:
                        pack(packs[k])
                    k += 1
            if "norm" in pend:
                normalize(*pend.pop("norm"))
            normalize(*drain_copies(*pend.pop("copies")))

            # ---- phase C: output projection ----
            for i in range(NT):
                y_tile(i)

            ps_o_cm.__exit__(None, None, None)
            ps_cm.__exit__(None, None, None)

    nc.compile()
    return nc


def _in_maps(x, context, mask, Wq, Wk, Wv, Wo):
    from ml_dtypes import bfloat16
    maps = []
    xb = np.asarray(x, dtype=np.float32).astype(bfloat16)
    cb = np.asarray(context, dtype=np.float32).astype(bfloat16)
    for core in range(N_CORES):
        b, hg = core // H_PER, core % H_PER
        c0 = hg * DHC
        maps.append({
            "xT": np.ascontiguousarray(xb[b].T),
            "cT": np.ascontiguousarray(cb[b].T),
            "msk": np.ascontiguousarray(
                np.asarray(mask[b]).astype(np.float32).reshape(M, 1)),
            "wq": np.ascontiguousarray(
                np.asarray(Wq[:, c0:c0 + DHC], dtype=np.float32)
                .astype(bfloat16)),
            "wk": np.ascontiguousarray(
                np.asarray(Wk[:, c0:c0 + DHC], dtype=np.float32)
                .astype(bfloat16)),
            "wv": np.ascontiguousarray(
                np.asarray(Wv[:, c0:c0 + DHC], dtype=np.float32)
                .astype(bfloat16)),
            "wo": np.ascontiguousarray(
                np.asarray(Wo[c0:c0 + DHC, :], dtype=np.float32)
                .astype(bfloat16)),
        })
    return maps


def _gather(results, bo):
    out = np.zeros((B, N, C), dtype=np.float32)
    for core in range(N_CORES):
        out[core // H_PER] += np.asarray(results[core]["y"],
                                         dtype=np.float32)
    out += np.asarray(bo, dtype=np.float32)
    return out


def kernel(x, context, mask, Wq, Wk, Wv, Wo, bo, **extra_kwargs):
    if "nc" not in _CACHE:
        _CACHE["nc"] = _build()
    nc = _CACHE["nc"]
    maps = _in_maps(x, context, mask, Wq, Wk, Wv, Wo)
    res = run_bass_kernel_spmd(nc, maps, core_ids=list(range(N_CORES)),
                               **extra_kwargs)
    out = _gather(res.results, bo)
    if extra_kwargs:
        _CACHE["last_result"] = res
    return out


# revision 24
# speedup vs baseline: 1.1129x; 1.1129x over previous
"""Cross-attention kernel for one TRN2 chip (8 NeuronCores).

Sharding: core = (batch b in {0,1}) x (head-group of 4 heads).  Each core
computes attention for its 4 heads of its batch element and a partial output
projection [N, 1024]; the host sums the 4 partials per batch and adds bias.

Key structure (all matmuls bf16, fp32 PSUM):
  - x/ctx cast-loaded to bf16, host-pretransposed; input DMA is ordered
    by first use (wk, mask, first ctx j-block, wv, ...) in whole-block
    issues so the first K projection starts ~10us earlier.
  - QK per m-tile: two concurrent row-tiled matmuls (head s0 on array rows
    0-63, s1 on rows 64-127) into one [128,1024] PSUM tile; ONE wide exp
    [128,1024] covers both heads; AV accumulates [65,512] per head with a
    ones-column producing the softmax denominator for free.
  - PSUM: sT double-buffered (4 banks) + oT double-buffered (4 banks) so
    the PE never stalls on the activation and HAM stays at 2.4 GHz.
  - exp split: 2 of 3 m-tiles on ScalarE native exp, 1 of 3 on DVE via an
    int16 Schraudolph fast-exp that writes bf16 in a single op.
  - Normalization: denominators are per-partition scalars in the natural
    [n, d] accumulation layout; reciprocal + tensor_scalar_mul.
  - Output y DMAs alternate between two issue engines so the tail is not
    serialized on the sync queue.
"""

import numpy as np

import concourse.bass as bass
import concourse.mybir as mybir
import concourse.tile as tile
from concourse import bacc
from concourse.masks import make_identity
from concourse.bass import ts
from concourse.bass_utils import run_bass_kernel_spmd
B, N, M, C = 2, 2048, 2048, 1024
HEADS, DH = 16, 64
H_PER = 4                # heads per core
DHC = H_PER * DH         # 256: per-core slice of INNER
SCALE = DH ** -0.5
P = 128
NT = N // P              # 16 n-tiles
MT = M // P              # 16 m-tiles
CCH = C // P             # 8 contraction chunks
FD = 512                 # attention n-chunk (PSUM bank)
NJ = N // FD             # 4 n-chunks
N_CORES = 8

F32 = mybir.dt.float32
BF16 = mybir.dt.bfloat16
I16 = mybir.dt.int16
EXP = mybir.ActivationFunctionType.Exp
# int16 Schraudolph fast-exp: exp(x*SCALE) ~= bitcast_bf16(i16(x*KS + BS))
KS16 = SCALE * (1 << 7) / float(np.log(2.0))
BS16 = float(127 * (1 << 7)) - 366392.0 / 65536.0

_CACHE = {}


def _build():
    nc = bacc.Bacc("TRN2", target_bir_lowering=False, debug=False,
                   num_devices=N_CORES, num_swdge_queues=4)

    xT_d = nc.dram_tensor("xT", (C, N), BF16, kind="ExternalInput").ap()
    cT_d = nc.dram_tensor("cT", (C, M), BF16, kind="ExternalInput").ap()
    msk_d = nc.dram_tensor("msk", (M, 1), F32, kind="ExternalInput").ap()
    wq_d = nc.dram_tensor("wq", (C, DHC), BF16, kind="ExternalInput").ap()
    wk_d = nc.dram_tensor("wk", (C, DHC), BF16, kind="ExternalInput").ap()
    wv_d = nc.dram_tensor("wv", (C, DHC), BF16, kind="ExternalInput").ap()
    wo_d = nc.dram_tensor("wo", (DHC, C), BF16, kind="ExternalInput").ap()
    y_d = nc.dram_tensor("y", (N, C), BF16, kind="ExternalOutput").ap()

    with tile.TileContext(nc) as tc:
        with (
            tc.tile_pool(name="const", bufs=1) as const,
            tc.tile_pool(name="stage", bufs=1) as stage,
            tc.tile_pool(name="pTp", bufs=3) as pTp,
            tc.tile_pool(name="drn", bufs=2) as drn,
            tc.tile_pool(name="norm", bufs=4) as norm,
            tc.tile_pool(name="yp", bufs=3) as yp,
        ):
            # ---- persistent SBUF tensors ----
            xT = const.tile([P, CCH, N], BF16, name="xT")
            cT = const.tile([P, CCH, M], BF16, name="cT")
            qT2 = const.tile([P, 2, N], BF16, name="qT2")
            kT2 = const.tile([P, 2, M], BF16, name="kT2")
            # v: [m-partition, m-tile, head, d(64)+ones(1)]
            v_sb = const.tile([P, MT, H_PER, DH + 1], BF16, name="v")
            wq_sb = const.tile([P, CCH, DHC], BF16, name="wq")
            wk_sb = const.tile([P, CCH, DHC], BF16, name="wk")
            wv_sb = const.tile([P, CCH, DHC], BF16, name="wv")
            wo_sb = const.tile([P, 2, C], BF16, name="wo")
            msk_sb = const.tile([P, MT, 1], F32, name="msk")
            oTn = const.tile([P, 2, N], BF16, name="oTn")

            # ---- input DMA, ordered by first use ----
            nc.sync.dma_start(
                out=wk_sb, in_=wk_d.rearrange("(cc p) d -> p cc d", p=P))
            nc.sync.dma_start(
                out=msk_sb, in_=msk_d.rearrange("(t p) o -> p t o", p=P))
            cTv = cT_d.rearrange("(cc p) n -> p cc n", p=P)
            xTv = xT_d.rearrange("(cc p) n -> p cc n", p=P)
            nc.sync.dma_start(
                out=cT[:, :, ts(0, FD)], in_=cTv[:, :, ts(0, FD)])
            nc.sync.dma_start(
                out=wv_sb, in_=wv_d.rearrange("(cc p) d -> p cc d", p=P))
            for g in range(1, 4):
                nc.sync.dma_start(
                    out=cT[:, :, ts(g, FD)], in_=cTv[:, :, ts(g, FD)])
            nc.sync.dma_start(
                out=wq_sb, in_=wq_d.rearrange("(cc p) d -> p cc d", p=P))
            for g in range(4):
                nc.sync.dma_start(
                    out=xT[:, :, ts(g, FD)], in_=xTv[:, :, ts(g, FD)])
            nc.sync.dma_start(
                out=wo_sb, in_=wo_d.rearrange("(dc p) e -> p dc e", p=P))

            nc.vector.memset(v_sb, 1.0)
            identf = stage.tile([P, P], F32, name="identf")
            make_identity(nc, identf)
            identb = const.tile([P, P], BF16, name="identb")
            nc.vector.tensor_copy(identb, identf)

            ps_p_cm = tc.tile_pool(name="ps_p", bufs=3, space="PSUM")
            ps_p = ps_p_cm.__enter__()       # [128,512] projections: 3 banks

            # project one n/m-chunk j of q or k (both d-chunks dc)
            def proj_T(w_sb, srcT, dstT2, dc, j, alt):
                ps = ps_p.tile([P, FD], F32, name="kq")
                for cc in range(CCH):
                    nc.tensor.matmul(
                        ps, lhsT=w_sb[:, cc, ts(dc, P)],
                        rhs=srcT[:, cc, ts(j, FD)],
                        start=(cc == 0), stop=(cc == CCH - 1))
                dst = dstT2[:, dc, ts(j, FD)]
                if alt:
                    nc.vector.tensor_copy(dst, ps)
                else:
                    nc.scalar.copy(dst, ps)

            # V projection for two m-tiles (one [128,512] PSUM tile)
            def proj_V(m0):
                vp = ps_p.tile([P, 2, DHC], F32, name="vp")
                for mi in range(2):
                    for cc in range(CCH):
                        nc.tensor.matmul(
                            vp[:, mi, :],
                            lhsT=cT[:, cc, ts(m0 + mi, P)],
                            rhs=wv_sb[:, cc, :],
                            start=(cc == 0), stop=(cc == CCH - 1))
                nc.vector.tensor_copy(
                    v_sb[:, m0:m0 + 2, :, 0:DH],
                    vp.rearrange("p mi (h d) -> p mi h d", h=H_PER))
                for mi in range(2):
                    nc.vector.tensor_scalar_mul(
                        v_sb[:, m0 + mi, :, :], v_sb[:, m0 + mi, :, :],
                        msk_sb[:, m0 + mi, :])

            # ---- phase A: project K/V/Q ----
            alt = 0
            for g in range(4):
                for dc in range(2):
                    proj_T(wk_sb, cT, kT2, dc, g, alt % 2)
                    alt += 1
                proj_V(4 * g)
                proj_V(4 * g + 2)
            for g in range(4):
                for dc in range(2):
                    proj_T(wq_sb, xT, qT2, dc, g, alt % 2)
                    alt += 1

            ps_p_cm.__exit__(None, None, None)

            # ---- phase B: attention (o accumulated in natural [n, d]
            # layout so the softmax denominator is a per-partition scalar) ----
            ps_s_cm = tc.tile_pool(name="ps_s", bufs=3, space="PSUM")
            ps_s = ps_s_cm.__enter__()       # [128,1024] scores: 6 banks
            ps_o_cm = tc.tile_pool(name="ps_o", bufs=1, space="PSUM")
            ps_o = ps_o_cm.__enter__()       # 2x[128,260] per j: 2 banks

            def qk(sT, dc, j, m):
                for s in range(2):
                    nc.tensor.matmul(
                        sT[:, s, :],
                        lhsT=kT2[s * DH:(s + 1) * DH, dc, ts(m, P)],
                        rhs=qT2[s * DH:(s + 1) * DH, dc, ts(j, FD)],
                        start=True, stop=True)

            def av(oPs, pT, dc, m):
                # o_nat[n, d] += pT[m, n]^T @ v[m, d|1]; stationary = pT chunk
                for s in range(2):
                    for sub in range(4):
                        nc.tensor.matmul(
                            oPs[s][:, sub, :],
                            lhsT=pT[:, s, ts(sub, P)],
                            rhs=v_sb[:, m, 2 * dc + s, :],
                            start=(m == 0 and sub == 0),
                            stop=(m == MT - 1),
                            skip_group_check=True)

            for dc in range(2):
                for j in range(NJ):
                    oPs = [ps_o.tile([P, 4, DH + 1], F32, name=f"o{s}")
                           for s in range(2)]
                    pTs = []
                    for m in range(MT):
                        sT = ps_s.tile([P, 2, FD], F32, name="sT")
                        qk(sT, dc, j, m)
                        if m >= 2:
                            av(oPs, pTs[m - 2], dc, m - 2)
                        pT = pTp.tile([P, 2, FD], BF16, name="pT")
                        if m % 3 == 2:
                            # DVE int16 Schraudolph fast-exp -> bf16,
                            # single op, offloads ScalarE
                            nc.vector.tensor_scalar(
                                pT.bitcast(I16)[:, :, :], sT, KS16, BS16,
                                op0=mybir.AluOpType.mult,
                                op1=mybir.AluOpType.add)
                        else:
                            nc.scalar.activation(pT, sT, EXP, scale=SCALE)
                        pTs.append(pT)
                    av(oPs, pTs[MT - 2], dc, MT - 2)
                    av(oPs, pTs[MT - 1], dc, MT - 1)
                    # drain + normalize: D is column 64 of each (s, sub) block
                    o_sb = drn.tile([P, 2, 4, DH + 1], F32, name="o_sb")
                    for s in range(2):
                        nc.vector.tensor_copy(o_sb[:, s], oPs[s])
                    rc = norm.tile([P, 2, 4, 1], F32, name="rc")
                    nc.vector.reciprocal(rc, o_sb[:, :, :, DH:DH + 1])
                    o_bf = drn.tile([P, 4, 2, DH], BF16, name="o_bf")
                    for s in range(2):
                        for sub in range(4):
                            nc.vector.tensor_scalar_mul(
                                o_bf[:, sub, s, :], o_sb[:, s, sub, 0:DH],
                                rc[:, s, sub, :])
                    # transpose back to d-major for the output projection
                    if dc == 1 and j == NJ - 1:
                        # last group: deferred PE transpose (runs after the
                        # already-ready y tiles so the PE queue never stalls)
                        last_obf = o_bf
                    else:
                        for sub in range(4):
                            nc.sync.dma_start_transpose(
                                out=oTn[:, dc, j * FD + sub * P:
                                        j * FD + (sub + 1) * P],
                                in_=o_bf[:, sub].rearrange("p s d -> p (s d)"))

            ps_o_cm.__exit__(None, None, None)
            ps_s_cm.__exit__(None, None, None)

            # ---- phase C: output projection ----
            ps_y_cm = tc.tile_pool(name="ps_y", bufs=3, space="PSUM")
            ps_y = ps_y_cm.__enter__()

            def y_tile(i):
                y_ps = ps_y.tile([P, C], F32, name="y")
                for col in range(2):
                    for dc in range(2):
                        nc.tensor.matmul(
                            y_ps[:, ts(col, FD)],
                            lhsT=oTn[:, dc, ts(i, P)],
                            rhs=wo_sb[:, dc, ts(col, FD)],
                            start=(dc == 0), stop=(dc == 1))
                y_sb = yp.tile([P, C], BF16, name="ysb")
                nc.vector.tensor_copy(y_sb[:, 0:FD], y_ps[:, 0:FD])
                nc.scalar.copy(y_sb[:, FD:C], y_ps[:, FD:C])
                # alternate the issue engine so the tail is not serialized
                if i % 2:
                    nc.gpsimd.dma_start(out=y_d[ts(i, P), :], in_=y_sb)
                else:
                    nc.sync.dma_start(out=y_d[ts(i, P), :], in_=y_sb)

            for i in range(NT - 4):
                y_tile(i)
            ps_t2_cm = tc.tile_pool(name="ps_t2", bufs=1, space="PSUM")
            ps_t2 = ps_t2_cm.__enter__()
            tp = ps_t2.tile([P, FD], F32, name="tpy")
            for sub in range(4):
                nc.tensor.matmul(
                    tp[:, ts(sub, P)],
                    lhsT=last_obf[:, sub].rearrange("p s d -> p (s d)"),
                    rhs=identb, start=True, stop=True)
            nc.vector.tensor_copy(oTn[:, 1, ts(NJ - 1, FD)], tp)
            for i in range(NT - 4, NT):
                y_tile(i)
            ps_t2_cm.__exit__(None, None, None)
            ps_y_cm.__exit__(None, None, None)

    nc.compile()
    return nc


def _in_maps(x, context, mask, Wq, Wk, Wv, Wo):
    from ml_dtypes import bfloat16
    maps = []
    xb = np.asarray(x, dtype=np.float32).astype(bfloat16)
    cb = np.asarray(context, dtype=np.float32).astype(bfloat16)
    for core in range(N_CORES):
        b, hg = core // H_PER, core % H_PER
        c0 = hg * DHC
        maps.append({
            "xT": np.ascontiguousarray(xb[b].T),
            "cT": np.ascontiguousarray(cb[b].T),
            "msk": np.ascontiguousarray(
                np.asarray(mask[b]).astype(np.float32).reshape(M, 1)),
            "wq": np.ascontiguousarray(
                np.asarray(Wq[:, c0:c0 + DHC], dtype=np.float32)
                .astype(bfloat16)),
            "wk": np.ascontiguousarray(
                np.asarray(Wk[:, c0:c0 + DHC], dtype=np.float32)
                .astype(bfloat16)),
            "wv": np.ascontiguousarray(
                np.asarray(Wv[:, c0:c0 + DHC], dtype=np.float32)
                .astype(bfloat16)),
            "wo": np.ascontiguousarray(
                np.asarray(Wo[c0:c0 + DHC, :], dtype=np.float32)
                .astype(bfloat16)),
        })
    return maps


def _gather(results, bo):
    out = np.zeros((B, N, C), dtype=np.float32)
    for core in range(N_CORES):
        out[core // H_PER] += np.asarray(results[core]["y"],
                                         dtype=np.float32)
    out += np.asarray(bo, dtype=np.float32)
    return out


def kernel(x, context, mask, Wq, Wk, Wv, Wo, bo, **extra_kwargs):
    if "nc" not in _CACHE:
        _CACHE["nc"] = _build()
    nc = _CACHE["nc"]
    maps = _in_maps(x, context, mask, Wq, Wk, Wv, Wo)
    res = run_bass_kernel_spmd(nc, maps, core_ids=list(range(N_CORES)),
                               **extra_kwargs)
    out = _gather(res.results, bo)
    if extra_kwargs:
        _CACHE["last_result"] = res
    return out


# revision 25
# speedup vs baseline: 1.1262x; 1.0119x over previous
"""Cross-attention kernel for one TRN2 chip (8 NeuronCores).

Sharding: core = (batch b in {0,1}) x (head-group of 4 heads).  Each core
computes attention for its 4 heads of its batch element and a partial output
projection [N, 1024]; the host sums the 4 partials per batch and adds bias.

Key structure (all matmuls bf16, fp32 PSUM):
  - x/ctx cast-loaded to bf16, host-pretransposed; input DMA is ordered
    by first use (wk, mask, first ctx j-block, wv, ...) in whole-block
    issues so the first K projection starts ~10us earlier.
  - QK per m-tile: two concurrent row-tiled matmuls (head s0 on array rows
    0-63, s1 on rows 64-127) into one [128,1024] PSUM tile; ONE wide exp
    [128,1024] covers both heads; AV accumulates [65,512] per head with a
    ones-column producing the softmax denominator for free.
  - PSUM: sT double-buffered (4 banks) + oT double-buffered (4 banks) so
    the PE never stalls on the activation and HAM stays at 2.4 GHz.
  - exp split: 2 of 3 m-tiles on ScalarE native exp, 1 of 3 on DVE via an
    int16 Schraudolph fast-exp that writes bf16 in a single op.
  - Normalization: denominators are per-partition scalars in the natural
    [n, d] accumulation layout; reciprocal + tensor_scalar_mul.
  - Output y DMAs alternate between two issue engines so the tail is not
    serialized on the sync queue.
"""

import numpy as np

import concourse.bass as bass
import concourse.mybir as mybir
import concourse.tile as tile
from concourse import bacc
from concourse.masks import make_identity
from concourse.bass import ts
from concourse.bass_utils import run_bass_kernel_spmd
B, N, M, C = 2, 2048, 2048, 1024
HEADS, DH = 16, 64
H_PER = 4                # heads per core
DHC = H_PER * DH         # 256: per-core slice of INNER
SCALE = DH ** -0.5
P = 128
NT = N // P              # 16 n-tiles
MT = M // P              # 16 m-tiles
CCH = C // P             # 8 contraction chunks
FD = 512                 # attention n-chunk (PSUM bank)
NJ = N // FD             # 4 n-chunks
N_CORES = 8

F32 = mybir.dt.float32
BF16 = mybir.dt.bfloat16
I16 = mybir.dt.int16
EXP = mybir.ActivationFunctionType.Exp
# int16 Schraudolph fast-exp: exp(x*SCALE) ~= bitcast_bf16(i16(x*KS + BS))
KS16 = SCALE * (1 << 7) / float(np.log(2.0))
BS16 = float(127 * (1 << 7)) - 366392.0 / 65536.0

_CACHE = {}


def _build():
    nc = bacc.Bacc("TRN2", target_bir_lowering=False, debug=False,
                   num_devices=N_CORES, num_swdge_queues=4)

    xT_d = nc.dram_tensor("xT", (C, N), BF16, kind="ExternalInput").ap()
    cT_d = nc.dram_tensor("cT", (C, M), BF16, kind="ExternalInput").ap()
    msk_d = nc.dram_tensor("msk", (M, 1), F32, kind="ExternalInput").ap()
    wq_d = nc.dram_tensor("wq", (C, DHC), BF16, kind="ExternalInput").ap()
    wk_d = nc.dram_tensor("wk", (C, DHC), BF16, kind="ExternalInput").ap()
    wv_d = nc.dram_tensor("wv", (C, DHC), BF16, kind="ExternalInput").ap()
    wo_d = nc.dram_tensor("wo", (DHC, C), BF16, kind="ExternalInput").ap()
    y_d = nc.dram_tensor("y", (N, C), BF16, kind="ExternalOutput").ap()

    with tile.TileContext(nc) as tc:
        with (
            tc.tile_pool(name="const", bufs=1) as const,
            tc.tile_pool(name="stage", bufs=1) as stage,
            tc.tile_pool(name="pTp", bufs=3) as pTp,
            tc.tile_pool(name="drn", bufs=2) as drn,
            tc.tile_pool(name="norm", bufs=4) as norm,
            tc.tile_pool(name="yp", bufs=3) as yp,
        ):
            # ---- persistent SBUF tensors ----
            xT = const.tile([P, CCH, N], BF16, name="xT")
            cT = const.tile([P, CCH, M], BF16, name="cT")
            qT2 = const.tile([P, 2, N], BF16, name="qT2")
            kT2 = const.tile([P, 2, M], BF16, name="kT2")
            # v: [m-partition, m-tile, head, d(64)+ones(1)]
            v_sb = const.tile([P, MT, H_PER, DH + 1], BF16, name="v")
            wq_sb = const.tile([P, CCH, DHC], BF16, name="wq")
            wk_sb = const.tile([P, CCH, DHC], BF16, name="wk")
            wv_sb = const.tile([P, CCH, DHC], BF16, name="wv")
            wo_sb = const.tile([P, 2, C], BF16, name="wo")
            msk_sb = const.tile([P, MT, 1], F32, name="msk")
            oTn = const.tile([P, 2, N], BF16, name="oTn")

            # ---- input DMA, ordered by first use ----
            cTv = cT_d.rearrange("(cc p) n -> p cc n", p=P)
            xTv = xT_d.rearrange("(cc p) n -> p cc n", p=P)
            nc.sync.dma_start(
                out=wv_sb, in_=wv_d.rearrange("(cc p) d -> p cc d", p=P))
            nc.sync.dma_start(
                out=msk_sb, in_=msk_d.rearrange("(t p) o -> p t o", p=P))
            nc.sync.dma_start(
                out=cT[:, :, 0:DHC], in_=cTv[:, :, 0:DHC])
            nc.sync.dma_start(
                out=cT[:, :, DHC:FD], in_=cTv[:, :, DHC:FD])
            nc.sync.dma_start(
                out=wk_sb, in_=wk_d.rearrange("(cc p) d -> p cc d", p=P))
            for g in range(1, 4):
                nc.sync.dma_start(
                    out=cT[:, :, ts(g, FD)], in_=cTv[:, :, ts(g, FD)])
            nc.sync.dma_start(
                out=wq_sb, in_=wq_d.rearrange("(cc p) d -> p cc d", p=P))
            for g in range(4):
                nc.sync.dma_start(
                    out=xT[:, :, ts(g, FD)], in_=xTv[:, :, ts(g, FD)])
            nc.sync.dma_start(
                out=wo_sb, in_=wo_d.rearrange("(dc p) e -> p dc e", p=P))

            nc.vector.memset(v_sb, 1.0)
            identf = stage.tile([P, P], F32, name="identf")
            make_identity(nc, identf)
            identb = const.tile([P, P], BF16, name="identb")
            nc.vector.tensor_copy(identb, identf)

            ps_p_cm = tc.tile_pool(name="ps_p", bufs=3, space="PSUM")
            ps_p = ps_p_cm.__enter__()       # [128,512] projections: 3 banks

            # project one n/m-chunk j of q or k (both d-chunks dc)
            def proj_T(w_sb, srcT, dstT2, dc, j, alt):
                ps = ps_p.tile([P, FD], F32, name="kq")
                for cc in range(CCH):
                    nc.tensor.matmul(
                        ps, lhsT=w_sb[:, cc, ts(dc, P)],
                        rhs=srcT[:, cc, ts(j, FD)],
                        start=(cc == 0), stop=(cc == CCH - 1))
                dst = dstT2[:, dc, ts(j, FD)]
                if alt:
                    nc.vector.tensor_copy(dst, ps)
                else:
                    nc.scalar.copy(dst, ps)

            # V projection for two m-tiles (one [128,512] PSUM tile)
            def proj_V(m0):
                vp = ps_p.tile([P, 2, DHC], F32, name="vp")
                for mi in range(2):
                    for cc in range(CCH):
                        nc.tensor.matmul(
                            vp[:, mi, :],
                            lhsT=cT[:, cc, ts(m0 + mi, P)],
                            rhs=wv_sb[:, cc, :],
                            start=(cc == 0), stop=(cc == CCH - 1))
                nc.vector.tensor_copy(
                    v_sb[:, m0:m0 + 2, :, 0:DH],
                    vp.rearrange("p mi (h d) -> p mi h d", h=H_PER))
                for mi in range(2):
                    nc.vector.tensor_scalar_mul(
                        v_sb[:, m0 + mi, :, :], v_sb[:, m0 + mi, :, :],
                        msk_sb[:, m0 + mi, :])

            # ---- phase A: project K/V/Q ----
            alt = 0
            for g in range(4):
                proj_V(4 * g)
                proj_V(4 * g + 2)
                for dc in range(2):
                    proj_T(wk_sb, cT, kT2, dc, g, alt % 2)
                    alt += 1
            for g in range(4):
                for dc in range(2):
                    proj_T(wq_sb, xT, qT2, dc, g, alt % 2)
                    alt += 1

            ps_p_cm.__exit__(None, None, None)

            # ---- phase B: attention (o accumulated in natural [n, d]
            # layout so the softmax denominator is a per-partition scalar) ----
            ps_s_cm = tc.tile_pool(name="ps_s", bufs=3, space="PSUM")
            ps_s = ps_s_cm.__enter__()       # [128,1024] scores: 6 banks
            ps_o_cm = tc.tile_pool(name="ps_o", bufs=1, space="PSUM")
            ps_o = ps_o_cm.__enter__()       # 2x[128,260] per j: 2 banks

            def qk(sT, dc, j, m):
                for s in range(2):
                    nc.tensor.matmul(
                        sT[:, s, :],
                        lhsT=kT2[s * DH:(s + 1) * DH, dc, ts(m, P)],
                        rhs=qT2[s * DH:(s + 1) * DH, dc, ts(j, FD)],
                        start=True, stop=True)

            def av(oPs, pT, dc, m):
                # o_nat[n, d] += pT[m, n]^T @ v[m, d|1]; stationary = pT chunk
                for s in range(2):
                    for sub in range(4):
                        nc.tensor.matmul(
                            oPs[s][:, sub, :],
                            lhsT=pT[:, s, ts(sub, P)],
                            rhs=v_sb[:, m, 2 * dc + s, :],
                            start=(m == 0 and sub == 0),
                            stop=(m == MT - 1),
                            skip_group_check=True)

            for dc in range(2):
                for j in range(NJ):
                    oPs = [ps_o.tile([P, 4, DH + 1], F32, name=f"o{s}")
                           for s in range(2)]
                    pTs = []
                    for m in range(MT):
                        sT = ps_s.tile([P, 2, FD], F32, name="sT")
                        qk(sT, dc, j, m)
                        if m >= 2:
                            av(oPs, pTs[m - 2], dc, m - 2)
                        pT = pTp.tile([P, 2, FD], BF16, name="pT")
                        if m % 3 == 2:
                            # DVE int16 Schraudolph fast-exp -> bf16,
                            # single op, offloads ScalarE
                            nc.vector.tensor_scalar(
                                pT.bitcast(I16)[:, :, :], sT, KS16, BS16,
                                op0=mybir.AluOpType.mult,
                                op1=mybir.AluOpType.add)
                        else:
                            nc.scalar.activation(pT, sT, EXP, scale=SCALE)
                        pTs.append(pT)
                    av(oPs, pTs[MT - 2], dc, MT - 2)
                    av(oPs, pTs[MT - 1], dc, MT - 1)
                    # drain + normalize: D is column 64 of each (s, sub) block
                    o_sb = drn.tile([P, 2, 4, DH + 1], F32, name="o_sb")
                    for s in range(2):
                        nc.vector.tensor_copy(o_sb[:, s], oPs[s])
                    rc = norm.tile([P, 2, 4, 1], F32, name="rc")
                    nc.vector.reciprocal(rc, o_sb[:, :, :, DH:DH + 1])
                    o_bf = drn.tile([P, 4, 2, DH], BF16, name="o_bf")
                    for s in range(2):
                        for sub in range(4):
                            nc.vector.tensor_scalar_mul(
                                o_bf[:, sub, s, :], o_sb[:, s, sub, 0:DH],
                                rc[:, s, sub, :])
                    # transpose back to d-major for the output projection
                    if dc == 1 and j == NJ - 1:
                        # last group: deferred PE transpose (runs after the
                        # already-ready y tiles so the PE queue never stalls)
                        last_obf = o_bf
                    else:
                        for sub in range(4):
                            nc.sync.dma_start_transpose(
                                out=oTn[:, dc, j * FD + sub * P:
                                        j * FD + (sub + 1) * P],
                                in_=o_bf[:, sub].rearrange("p s d -> p (s d)"))

            ps_o_cm.__exit__(None, None, None)
            ps_s_cm.__exit__(None, None, None)

            # ---- phase C: output projection ----
            ps_y_cm = tc.tile_pool(name="ps_y", bufs=3, space="PSUM")
            ps_y = ps_y_cm.__enter__()

            def y_tile(i):
                y_ps = ps_y.tile([P, C], F32, name="y")
                for col in range(2):
                    for dc in range(2):
                        nc.tensor.matmul(
                            y_ps[:, ts(col, FD)],
                            lhsT=oTn[:, dc, ts(i, P)],
                            rhs=wo_sb[:, dc, ts(col, FD)],
                            start=(dc == 0), stop=(dc == 1))
                y_sb = yp.tile([P, C], BF16, name="ysb")
                nc.vector.tensor_copy(y_sb[:, 0:FD], y_ps[:, 0:FD])
                nc.scalar.copy(y_sb[:, FD:C], y_ps[:, FD:C])
                # alternate the issue engine so the tail is not serialized
                if i % 2:
                    nc.gpsimd.dma_start(out=y_d[ts(i, P), :], in_=y_sb)
                else:
                    nc.sync.dma_start(out=y_d[ts(i, P), :], in_=y_sb)

            ps_t2_cm = tc.tile_pool(name="ps_t2", bufs=1, space="PSUM")
            ps_t2 = ps_t2_cm.__enter__()
            tp = ps_t2.tile([P, FD], F32, name="tpy")
            for sub in range(4):
                nc.tensor.matmul(
                    tp[:, ts(sub, P)],
                    lhsT=last_obf[:, sub].rearrange("p s d -> p (s d)"),
                    rhs=identb, start=True, stop=True)
            nc.vector.tensor_copy(oTn[:, 1, ts(NJ - 1, FD)], tp)
            for i in range(NT):
                y_tile(i)
            ps_t2_cm.__exit__(None, None, None)
            ps_y_cm.__exit__(None, None, None)

    nc.compile()
    return nc


def _in_maps(x, context, mask, Wq, Wk, Wv, Wo):
    from ml_dtypes import bfloat16
    maps = []
    xb = np.asarray(x, dtype=np.float32).astype(bfloat16)
    cb = np.asarray(context, dtype=np.float32).astype(bfloat16)
    for core in range(N_CORES):
        b, hg = core // H_PER, core % H_PER
        c0 = hg * DHC
        maps.append({
            "xT": np.ascontiguousarray(xb[b].T),
            "cT": np.ascontiguousarray(cb[b].T),
            "msk": np.ascontiguousarray(
                np.asarray(mask[b]).astype(np.float32).reshape(M, 1)),
            "wq": np.ascontiguousarray(
                np.asarray(Wq[:, c0:c0 + DHC], dtype=np.float32)
                .astype(bfloat16)),
            "wk": np.ascontiguousarray(
                np.asarray(Wk[:, c0:c0 + DHC], dtype=np.float32)
                .astype(bfloat16)),
            "wv": np.ascontiguousarray(
                np.asarray(Wv[:, c0:c0 + DHC], dtype=np.float32)
                .astype(bfloat16)),
            "wo": np.ascontiguousarray(
                np.asarray(Wo[c0:c0 + DHC, :], dtype=np.float32)
                .astype(bfloat16)),
        })
    return maps


def _gather(results, bo):
    out = np.zeros((B, N, C), dtype=np.float32)
    for core in range(N_CORES):
        out[core // H_PER] += np.asarray(results[core]["y"],
                                         dtype=np.float32)
    out += np.asarray(bo, dtype=np.float32)
    return out


def kernel(x, context, mask, Wq, Wk, Wv, Wo, bo, **extra_kwargs):
    if "nc" not in _CACHE:
        _CACHE["nc"] = _build()
    nc = _CACHE["nc"]
    maps = _in_maps(x, context, mask, Wq, Wk, Wv, Wo)
    res = run_bass_kernel_spmd(nc, maps, core_ids=list(range(N_CORES)),
                               **extra_kwargs)
    out = _gather(res.results, bo)
    if extra_kwargs:
        _CACHE["last_result"] = res
    return out
